# revision 1
# baseline (speedup 1.0000x reference)
"""Autoformer attention block kernel for 8 TRN2 NeuronCores.

Math reduction (validated vs reference to 2e-7):
 - output = x + AutoCorrelation(series_decomp(LN(x)))  (final decomp s2+t2 == x2)
 - mean over lags of the FFT cross-correlation == (sum_t Q)*(sum_t K)  (DC bin),
   so no FFT is needed: top-k stats come from column sums of `seasonal`.
 - beta cancels exactly (band operator has row-sum 1); gamma folds into
   Wvo = diag(gamma) @ Wv @ Wo and the qsum scaling.
 - delay aggregation = 64-tap circular FIR along time with data-dependent
   weights -> banded Toeplitz matmul on the TensorEngine.

Sharding: data-parallel over batch (B=8 -> 8 cores); one [64]-float AllReduce
for the global top-40 channel selection.
"""

import sys

if "/opt/trn_rl_repo" not in sys.path:
    sys.path.insert(0, "/opt/trn_rl_repo")

import numpy as np

L = 3072
D = 512
NT = L // 128  # 24 time tiles
H = 8
DK = 64
KTOP = 40
PAD = 12  # (25-1)//2
EPS = 1e-5
NCORES = 8
HL = float(H * L)

_CACHE = {}


def _np_consts():
    t = np.arange(L)
    lo = np.maximum(t - PAD, 0)
    hi = np.minimum(t + PAD + 1, L)
    inv = 1.0 / (hi - lo).astype(np.float64)

    # phi[s] = 1 - sum over t in the window around s of 1/win(t); nonzero only
    # in the first/last 24 positions.
    phi = np.ones(L, np.float64)
    for s in range(L):
        a = max(0, s - PAD)
        b = min(L, s + PAD + 1)
        phi[s] -= inv[a:b].sum()

    # band lhsT consts, all [128,128], K = a full z tile, zero-padded:
    # chunk X in {A: s = t0-128+j, B: s = t0+j, C: s = t0+128+j}:
    #   M[j, p] = delta(s, t0+p) - [|t0+p - s| <= PAD] / win(t0+p)
    def band(t0, soff):
        j = np.arange(128)[:, None]
        p = np.arange(128)[None, :]
        s = soff + j
        tp = t0 + p
        m = (np.abs(tp - s) <= PAD) & (s >= 0) & (s < L)
        M = -(m * inv[np.clip(tp, 0, L - 1)])
        M = M + (s == tp) * 1.0
        return np.ascontiguousarray(M, np.float32)

    t0m = 1280  # any interior tile
    b_A = band(t0m, t0m - 128)
    b_C = band(t0m, t0m + 128)
    b_Bf = band(0, 0)
    b_Bm = band(t0m, t0m)
    b_Bl = band(L - 128, L - 128)
    phi48 = np.zeros((128, 1), np.float32)
    phi48[:24, 0] = phi[:24]
    phi48[64:88, 0] = phi[-24:]
    ident = np.eye(128, dtype=np.float32)
    return b_A, b_C, b_Bf, b_Bm, b_Bl, phi48, ident


def _build():
    import concourse.bass as bass
    import concourse.tile as tile
    import concourse.mybir as mybir
    from concourse import bacc
    import bass_rust
    import ml_dtypes

    dt = mybir.dt
    f32 = dt.float32
    bf16 = dt.bfloat16
    AF = mybir.ActivationFunctionType
    ALU = mybir.AluOpType
    AX = mybir.AxisListType
    ts = bass.ts

    nc = bacc.Bacc(None, target_bir_lowering=False)

    xe = nc.dram_tensor("xb", [L, D], f32, kind="ExternalInput")
    wqe = nc.dram_tensor("Wq", [D, D], f32, kind="ExternalInput")
    wke = nc.dram_tensor("Wk", [D, D], f32, kind="ExternalInput")
    wve = nc.dram_tensor("Wv", [D, D], f32, kind="ExternalInput")
    woe = nc.dram_tensor("Wo", [D, D], f32, kind="ExternalInput")
    bqe = nc.dram_tensor("bq", [D], f32, kind="ExternalInput")
    bke = nc.dram_tensor("bk", [D], f32, kind="ExternalInput")
    bve = nc.dram_tensor("bv", [D], f32, kind="ExternalInput")
    boe = nc.dram_tensor("bo", [D], f32, kind="ExternalInput")
    gme = nc.dram_tensor("gamma", [D], f32, kind="ExternalInput")
    oute = nc.dram_tensor("out", [L, D], f32, kind="ExternalOutput")

    bA, bC, bBf, bBm, bBl, phi48, ident = _np_consts()
    bf = ml_dtypes.bfloat16
    cbA = nc.inline_tensor(bA.astype(bf), "c_bA")
    cbC = nc.inline_tensor(bC.astype(bf), "c_bC")
    cbBf = nc.inline_tensor(bBf.astype(bf), "c_bBf")
    cbBm = nc.inline_tensor(bBm.astype(bf), "c_bBm")
    cbBl = nc.inline_tensor(bBl.astype(bf), "c_bBl")
    cphi = nc.inline_tensor(phi48.astype(bf), "c_phi")
    cid = nc.inline_tensor(ident.astype(bf), "c_id")
    cones1x64 = nc.inline_tensor(np.ones((1, 64), np.float32), "c_o64")
    cones1x128b = nc.inline_tensor(np.ones((1, 128), bf), "c_o128b")

    from contextlib import ExitStack

    with tile.TileContext(nc) as tc, ExitStack() as ctx:
        pc = ctx.enter_context(tc.tile_pool(name="consts", bufs=1))
        px = ctx.enter_context(tc.tile_pool(name="xarr", bufs=NT))
        pz = ctx.enter_context(tc.tile_pool(name="zroll", bufs=10))
        pvo = ctx.enter_context(tc.tile_pool(name="voarr", bufs=NT))
        pwvo = ctx.enter_context(tc.tile_pool(name="wvo", bufs=4))
        pwt = ctx.enter_context(tc.tile_pool(name="wtmp", bufs=4))
        pwork = ctx.enter_context(tc.tile_pool(name="work", bufs=3))
        psq = ctx.enter_context(tc.tile_pool(name="sqscr", bufs=2))
        pstt = ctx.enter_context(tc.tile_pool(name="stats", bufs=3))
        psm = ctx.enter_context(tc.tile_pool(name="smalls", bufs=2))
        pout = ctx.enter_context(tc.tile_pool(name="osb", bufs=3))
        pseasT = ctx.enter_context(tc.tile_pool(name="seasT", bufs=3))
        pdram = ctx.enter_context(tc.tile_pool(name="dram", bufs=1, space="DRAM"))
        qst = ctx.enter_context(tc.tile_pool(name="ps_st", bufs=1, space="PSUM"))
        qtp = ctx.enter_context(tc.tile_pool(name="ps_tp", bufs=1, space="PSUM"))
        qvo = ctx.enter_context(tc.tile_pool(name="ps_vo", bufs=2, space="PSUM"))
        qsm = ctx.enter_context(tc.tile_pool(name="ps_sm", bufs=2, space="PSUM"))
        qtap = ctx.enter_context(tc.tile_pool(name="ps_tap", bufs=2, space="PSUM"))

        # ---------------- constants to SBUF ----------------
        def cload(name, shape, src, dtype=f32):
            t = pc.tile(list(shape), dtype, tag=name)
            nc.sync.dma_start(t[:], src)
            return t

        idt = cload("idt", (128, 128), cid[:, :], bf16)
        gammaP = pc.tile([128, 4], f32, tag="gammaP")
        nc.sync.dma_start(gammaP[:], gme[:].rearrange("(a b) -> b a", b=128))
        bndA = cload("bndA", (128, 128), cbA[:, :], bf16)
        bndC = cload("bndC", (128, 128), cbC[:, :], bf16)
        bndBf = cload("bndBf", (128, 128), cbBf[:, :], bf16)
        bndBm = cload("bndBm", (128, 128), cbBm[:, :], bf16)
        bndBl = cload("bndBl", (128, 128), cbBl[:, :], bf16)
        phis = cload("phis", (128, 1), cphi[:, :], bf16)
        o1x64 = cload("o1x64", (1, 64), cones1x64[:, :])
        o1x128b = cload("o1x128b", (1, 128), cones1x128b[:, :], bf16)
        bvP = pc.tile([128, 4], f32, tag="bvP")
        nc.sync.dma_start(bvP[:], bve[:].rearrange("(a b) -> b a", b=128))
        bqv = pc.tile([1, 512], f32, tag="bqv")
        nc.sync.dma_start(bqv[:], bqe[:])
        bkv = pc.tile([1, 512], f32, tag="bkv")
        nc.sync.dma_start(bkv[:], bke[:])
        bov = pc.tile([1, 512], f32, tag="bov")
        nc.sync.dma_start(bov[:], boe[:])
        bq_sc = pc.tile([1, 512], f32, tag="bq_sc")
        nc.scalar.mul(bq_sc[:], bqv[:], float(L))
        bk_sc = pc.tile([1, 512], f32, tag="bk_sc")
        nc.scalar.mul(bk_sc[:], bkv[:], float(L))

        ones64 = nc.const_aps.tensor(1.0, (64, 1))

        # toeplitz scratch in DRAM ([128 x 192] p-major), zeroed early
        toep2d = pdram.tile([128, 192], bf16, tag="toep2d")
        zline = pc.tile([128, 192], bf16, tag="zline")
        nc.vector.memset(zline[:], 0.0)
        nc.sync.dma_start(toep2d[:], zline[:])
        wfd = pdram.tile([64], bf16, tag="wfd")

        # ---------------- x tiles + grouped LN stats ----------------
        xt = [None] * NT
        zt = [None] * NT

        def emit_group(tiles):
            st = pstt.tile([128, 36], f32, tag="st")
            n = len(tiles)
            for j, i in enumerate(tiles):
                x = px.tile([128, 512], f32, tag="x")
                nc.sync.dma_start(x[:], xe[ts(i, 128), :])
                xt[i] = x
                nc.vector.tensor_reduce(
                    st[:, j : j + 1], x[:], axis=AX.X, op=ALU.add
                )
                sq = psq.tile([128, 512], f32, tag="sq")
                nc.scalar.activation(
                    sq[:], x[:], AF.Square, accum_out=st[:, 4 + j : 5 + j]
                )
            # mu = sx/D; mu2 = mu^2; t2 = sxx/D + eps; var = t2 - mu2
            # sd = sqrt(var); r = 1/sd; nmr = -(mu*r)
            nc.vector.tensor_scalar(
                st[:, 8 : 8 + n], st[:, 0:n], 1.0 / D, None, op0=ALU.mult
            )
            nc.vector.tensor_tensor(
                st[:, 12 : 12 + n], st[:, 8 : 8 + n], st[:, 8 : 8 + n], op=ALU.mult
            )
            nc.vector.tensor_scalar(
                st[:, 16 : 16 + n], st[:, 4 : 4 + n], 1.0 / D, EPS,
                op0=ALU.mult, op1=ALU.add,
            )
            nc.vector.tensor_tensor(
                st[:, 20 : 20 + n], st[:, 16 : 16 + n], st[:, 12 : 12 + n],
                op=ALU.subtract,
            )
            nc.scalar.activation(st[:, 24 : 24 + n], st[:, 20 : 20 + n], AF.Sqrt)
            nc.vector.reciprocal(st[:, 28 : 28 + n], st[:, 24 : 24 + n])
            nc.vector.tensor_tensor(
                st[:, 32 : 32 + n], st[:, 8 : 8 + n], st[:, 28 : 28 + n],
                op=ALU.mult,
            )
            nc.vector.tensor_scalar(
                st[:, 32 : 32 + n], st[:, 32 : 32 + n], -1.0, None, op0=ALU.mult
            )
            for j, i in enumerate(tiles):
                z = pz.tile([128, 512], bf16, tag="z")
                nc.scalar.activation(
                    z[:], xt[i][:], AF.Identity,
                    bias=st[:, 32 + j : 33 + j], scale=st[:, 28 + j : 29 + j],
                )
                zt[i] = z

        # ---------------- seasonal (banded matmul) + vo ----------------
        vo = [None] * NT
        wvo = []  # filled by weight prep below
        toep_ref = {}

        def emit_seasonal(i):
            sps = qst.tile([128, 512], f32)
            if i == 0:
                chunks = [(bndBf[:], zt[0][:, :]), (bndC[:], zt[1][:, :])]
            elif i == NT - 1:
                chunks = [(bndA[:], zt[22][:, :]), (bndBl[:], zt[23][:, :])]
            else:
                chunks = [
                    (bndA[:], zt[i - 1][:, :]),
                    (bndBm[:], zt[i][:, :]),
                    (bndC[:], zt[i + 1][:, :]),
                ]
            nck = len(chunks)
            for k, (lt, rz) in enumerate(chunks):
                nc.tensor.matmul(
                    sps[:], lt, rz, start=(k == 0), stop=(k == nck - 1)
                )
            seas = pwork.tile([128, 512], bf16, tag="seas")
            nc.scalar.copy(seas[:], sps[:])
            tp = qtp.tile([128, 512], bf16, tag="tp")
            for c in range(4):
                nc.tensor.transpose(tp[:, ts(c, 128)], seas[:, ts(c, 128)], idt[:])
            sT = pseasT.tile([128, 512], bf16, tag="sT")
            nc.vector.tensor_copy(sT[:], tp[:])
            vps = qvo.tile([128, 512], f32)
            for c in range(4):
                nc.tensor.matmul(
                    vps[:], sT[:, ts(c, 128)], wvo[c][:],
                    start=(c == 0), stop=(c == 3),
                )
            v = pvo.tile([128, 512], bf16, tag="vo")
            nc.scalar.copy(v[:], vps[:])
            vo[i] = v

        # ---------------- tap + residual + output ----------------
        def emit_tap(i):
            toepA = toep_ref["A"]
            toepB = toep_ref["B"]
            cvb = toep_ref["cvb"]
            tps = qtap.tile([128, 512], f32)
            nc.tensor.matmul(tps[:], toepA[:], vo[i][:], start=True, stop=False)
            nc.tensor.matmul(
                tps[:], toepB[:], vo[(i + 1) % NT][0:63, :],
                start=False, stop=False,
            )
            nc.tensor.matmul(tps[:], o1x128b[:], cvb[:], start=False, stop=True)
            osb = pout.tile([128, 512], f32, tag="osb")
            nc.vector.tensor_tensor(osb[:], xt[i][:], tps[:], op=ALU.add)
            if i % 2 == 0:
                nc.scalar.dma_start(oute[ts(i, 128), :], osb[:])
            else:
                nc.sync.dma_start(oute[ts(i, 128), :], osb[:])

        # ---------------- early qsum from the 48 boundary rows ----------------
        # phi is nonzero only on rows [0:24) and [L-24:L); LN is row-wise, so
        # compute z for just those rows in a dedicated tile (head at partition
        # 0, tail at partition 64 to satisfy matmul base-partition rules) and
        # feed the collective ~15us earlier than the full tiles would.
        with tc.high_priority():
            x48 = pwork.tile([128, 512], f32, tag="x48")
            nc.sync.dma_start(x48[0:24, :], xe[0:24, :])
            nc.sync.dma_start(x48[64:88, :], xe[L - 24 : L, :])
            st8 = pstt.tile([128, 36], f32, tag="st")
            nc.vector.tensor_reduce(
                st8[0:88, 0:1], x48[0:88, :], axis=AX.X, op=ALU.add
            )
            sq8 = psq.tile([128, 512], f32, tag="sq")
            nc.scalar.activation(
                sq8[0:88, :], x48[0:88, :], AF.Square,
                accum_out=st8[0:88, 1:2],
            )
            nc.vector.tensor_scalar(
                st8[0:88, 2:3], st8[0:88, 0:1], 1.0 / D, None, op0=ALU.mult
            )
            nc.vector.tensor_tensor(
                st8[0:88, 3:4], st8[0:88, 2:3], st8[0:88, 2:3], op=ALU.mult
            )
            nc.vector.tensor_scalar(
                st8[0:88, 4:5], st8[0:88, 1:2], 1.0 / D, EPS,
                op0=ALU.mult, op1=ALU.add,
            )
            nc.vector.tensor_tensor(
                st8[0:88, 5:6], st8[0:88, 4:5], st8[0:88, 3:4], op=ALU.subtract
            )
            nc.scalar.activation(st8[0:88, 6:7], st8[0:88, 5:6], AF.Sqrt)
            nc.vector.reciprocal(st8[0:88, 7:8], st8[0:88, 6:7])
            nc.vector.tensor_tensor(
                st8[0:88, 8:9], st8[0:88, 2:3], st8[0:88, 7:8], op=ALU.mult
            )
            nc.vector.tensor_scalar(
                st8[0:88, 9:10], st8[0:88, 8:9], -1.0, None, op0=ALU.mult
            )
            z48 = pwork.tile([128, 512], bf16, tag="z48")
            nc.scalar.activation(
                z48[0:88, :], x48[0:88, :], AF.Identity,
                bias=st8[0:88, 9:10], scale=st8[0:88, 7:8],
            )
            qps = qsm.tile([128, 8], f32, tag="sm")
            for c in range(4):
                nc.tensor.matmul(
                    qps[:, c : c + 1], z48[0:24, ts(c, 128)], phis[0:24, :],
                    start=True, stop=True,
                )
            for c in range(4):
                nc.tensor.matmul(
                    qps[:, 4 + c : 5 + c], z48[64:88, ts(c, 128)],
                    phis[64:88, :], start=True, stop=True,
                )

        # ---------------- weight prep: Wvo = diag(gamma) Wv Wo, cvec ----------------
        wo_sb = []
        for a in range(4):
            w = pwt.tile([128, 512], f32, tag="wo")
            nc.sync.dma_start(w[:], woe[ts(a, 128), :])
            wo_sb.append(w)
        wob = []
        for a in range(4):
            w = pwt.tile([128, 512], bf16, tag="wob")
            nc.vector.tensor_copy(w[:], wo_sb[a][:])
            wob.append(w)
        wv_sc = []
        for a in range(4):
            w = pwt.tile([128, 512], f32, tag="wv")
            nc.sync.dma_start(w[:], wve[ts(a, 128), :])
            ws = pwt.tile([128, 512], bf16, tag="wvs")
            nc.scalar.activation(ws[:], w[:], AF.Identity, scale=gammaP[:, a : a + 1])
            wv_sc.append(ws)
        wvT = []
        for c in range(4):
            w = pwt.tile([128, 512], bf16, tag="wvT")
            wvT.append(w)
        for a in range(4):
            for c in range(4):
                tp = qtp.tile([128, 128], bf16, tag="tp")
                nc.tensor.transpose(tp[:], wv_sc[a][:, ts(c, 128)], idt[:])
                nc.vector.tensor_copy(wvT[c][:, ts(a, 128)], tp[:])
        for a in range(4):
            vps = qvo.tile([128, 512], f32)
            for c in range(4):
                nc.tensor.matmul(
                    vps[:], wvT[c][:, ts(a, 128)], wob[c][:],
                    start=(c == 0), stop=(c == 3),
                )
            w = pwvo.tile([128, 512], bf16, tag="wvo")
            nc.scalar.copy(w[:], vps[:])
            wvo.append(w)

        # cvec = bv @ Wo + bo (bf16 row for the tap matmul)
        cps = qsm.tile([1, 512], f32, tag="sm")
        for c in range(4):
            nc.tensor.matmul(
                cps[:], bvP[:, c : c + 1], wo_sb[c][:],
                start=(c == 0), stop=(c == 3),
            )
        cv_sb = psm.tile([1, 512], f32, tag="cv")
        nc.vector.tensor_tensor(cv_sb[:], cps[:], bov[:], op=ALU.add)
        cvb = psm.tile([1, 512], bf16, tag="cvb")
        nc.vector.tensor_copy(cvb[:], cv_sb[:])
        toep_ref["cvb"] = cvb

        # ---------------- mv -> collective -> softmax weights -> toeplitz ----------------
        with tc.high_priority():
            wq_sb = []
            wk_sb = []
            for a in range(4):
                w = pwt.tile([128, 512], f32, tag="wq")
                nc.sync.dma_start(w[:], wqe[ts(a, 128), :])
                wb = pwt.tile([128, 512], bf16, tag="wqb")
                nc.vector.tensor_copy(wb[:], w[:])
                wq_sb.append(wb)
                w = pwt.tile([128, 512], f32, tag="wk")
                nc.sync.dma_start(w[:], wke[ts(a, 128), :])
                wb = pwt.tile([128, 512], bf16, tag="wkb")
                nc.vector.tensor_copy(wb[:], w[:])
                wk_sb.append(wb)

            qs_t = psm.tile([128, 4], f32, tag="qst")
            nc.scalar.copy(qs_t[:], qps[:, 4:8])
            qs_g = psm.tile([128, 4], bf16, tag="qsg")
            nc.vector.tensor_tensor(qs_g[:], qps[:, 0:4], qs_t[:], op=ALU.add)
            gb = psm.tile([128, 4], bf16, tag="gb")
            nc.vector.tensor_copy(gb[:], gammaP[:])
            nc.vector.tensor_tensor(qs_g[:], qs_g[:], gb[:], op=ALU.mult)

            qs_ps = qsm.tile([1, 512], f32, tag="sm")
            for c in range(4):
                nc.tensor.matmul(
                    qs_ps[:], qs_g[:, c : c + 1], wq_sb[c][:],
                    start=(c == 0), stop=(c == 3),
                )
            qsv = psm.tile([1, 512], f32, tag="qsv")
            nc.vector.tensor_tensor(qsv[:], qs_ps[:], bq_sc[:], op=ALU.add)
            ks_ps = qsm.tile([1, 512], f32, tag="sm")
            for c in range(4):
                nc.tensor.matmul(
                    ks_ps[:], qs_g[:, c : c + 1], wk_sb[c][:],
                    start=(c == 0), stop=(c == 3),
                )
            ksv = psm.tile([1, 512], f32, tag="ksv")
            nc.vector.tensor_tensor(ksv[:], ks_ps[:], bk_sc[:], op=ALU.add)

            pr = psm.tile([1, 512], f32, tag="pr")
            nc.vector.tensor_tensor(pr[:], qsv[:], ksv[:], op=ALU.mult)
            mvr = psm.tile([1, 64], f32, tag="mvr")
            nc.vector.tensor_reduce(
                mvr[:], pr[:].rearrange("p (h c) -> p c h", h=H),
                axis=AX.X, op=ALU.add,
            )
            mv = psm.tile([1, 64], f32, tag="mv")
            nc.scalar.mul(mv[:], mvr[:], 1.0 / HL)

            ccin = pdram.tile([64], f32, tag="ccin")
            ccout = pdram.tile([8, 64], f32, tag="ccout")
            mvd = pdram.tile([64], f32, tag="mvd")
            nc.gpsimd.dma_start(ccin[:], mv[:])
            nc.gpsimd.collective_compute(
                "AllGather",
                ALU.bypass,
                replica_groups=[list(range(NCORES))],
                ins=[ccin[:].opt()],
                outs=[ccout[:].opt()],
            )

        # ---------------- main pipeline ----------------
        groups = [[0, 1, 2, 3], [4, 5, 6, 7], [8, 9, 10, 11],
                  [12, 13, 14, 15], [16, 17, 18, 19]]
        state = {"sea": 0, "tap": 0}

        def advance():
            while state["sea"] < NT:
                i = state["sea"]
                need = [n for n in (i - 1, i, i + 1) if 0 <= n < NT]
                if not all(zt[n] is not None for n in need):
                    break
                emit_seasonal(i)
                state["sea"] += 1

        for g in groups:
            emit_group(g)
            advance()
        # post-collective: global mask, softmax weights, toeplitz build
        nc.gpsimd.dma_start(mvd[:], mv[:])
        g8 = psm.tile([1, 512], f32, tag="g8")
        nc.sync.dma_start(g8[:], ccout[:])
        g_row = psm.tile([1, 64], f32, tag="grow")
        nc.vector.tensor_reduce(
            g_row[:], g8[:].rearrange("p (r c) -> p c r", r=NCORES),
            axis=AX.X, op=ALU.add,
        )
        gP_ps = qsm.tile([64, 1], f32, tag="sm")
        nc.tensor.matmul(
            gP_ps[:], g_row[:], nc.const_aps.tensor(1.0, (1, 1)),
            start=True, stop=True,
        )
        gP = psm.tile([64, 1], f32, tag="gP")
        nc.vector.tensor_copy(gP[:], gP_ps[:])
        mvP = psm.tile([64, 1], f32, tag="mvP")
        nc.sync.dma_start(mvP[:], mvd[:])

        gf_ps = qsm.tile([64, 64], f32, tag="sm")
        nc.tensor.matmul(gf_ps[:], o1x64[:], g_row[:], start=True, stop=True)
        sc = psm.tile([64, 8], f32, tag="scm")
        cmp = psm.tile([64, 64], f32, tag="cmp")
        nc.vector.tensor_tensor(
            cmp[:], gf_ps[:], gP[:].to_broadcast((64, 64)), op=ALU.is_gt
        )
        nc.vector.tensor_reduce(sc[:, 0:1], cmp[:], axis=AX.X, op=ALU.add)
        nc.vector.tensor_scalar(
            sc[:, 1:2], sc[:, 0:1], KTOP - 0.5, None, op0=ALU.is_lt
        )
        nc.scalar.activation(sc[:, 2:3], mvP[:], AF.Exp)
        nc.vector.tensor_tensor(sc[:, 3:4], sc[:, 2:3], sc[:, 1:2], op=ALU.mult)
        s_ps = qsm.tile([1, 1], f32, tag="sm")
        nc.tensor.matmul(s_ps[:], sc[:, 3:4], ones64, start=True, stop=True)
        rs = psm.tile([1, 1], f32, tag="rs")
        nc.vector.reciprocal(rs[:], s_ps[:])
        rsf_ps = qsm.tile([64, 1], f32, tag="sm")
        nc.tensor.matmul(rsf_ps[:], o1x64[:], rs[:], start=True, stop=True)
        wf = psm.tile([64, 1], f32, tag="wf")
        nc.vector.tensor_tensor(wf[:], sc[:, 3:4], rsf_ps[:], op=ALU.mult)
        wfb = psm.tile([64, 1], bf16, tag="wfb")
        nc.vector.tensor_copy(wfb[:], wf[:])

        # toeplitz build: wf -> DRAM -> burst scatter (64-elem runs) ->
        # [p, j] tile -> PE transpose -> lhsT tiles toepA/toepB
        nc.sync.dma_start(wfd[:], wfb[:])
        dst = toep2d[:].flatten()
        dst.ap = bass_rust.VecI64Pair([[193, 128], [1, 64]])
        src = wfd[:].flatten()
        src.ap = bass_rust.VecI64Pair([[0, 128], [1, 64]])
        nc.sync.dma_start(dst, src)
        t2sb = pc.tile([128, 192], bf16, tag="t2sb")
        nc.sync.dma_start(t2sb[:], toep2d[:])
        tpa = qtp.tile([128, 128], bf16, tag="tp")
        nc.tensor.transpose(tpa[:], t2sb[:, 0:128], idt[:])
        toepA = pc.tile([128, 128], bf16, tag="toepA")
        nc.vector.tensor_copy(toepA[:], tpa[:])
        tpb = qtp.tile([128, 128], bf16, tag="tp")
        nc.tensor.transpose(tpb[0:63, :], t2sb[:, 128:191], idt[:])
        toepB = pc.tile([63, 128], bf16, tag="toepB")
        nc.vector.tensor_copy(toepB[:], tpb[0:63, :])
        toep_ref["A"] = toepA
        toep_ref["B"] = toepB
        # taps for the already-computed tiles; by the time the PE queue
        # reaches these, the collective (started ~17us) has completed, so
        # they do not jam the in-order engine queues.
        for i in range(17):
            emit_tap(i)
        emit_group([20, 21, 22, 23])
        advance()
        for i in range(17, NT):
            emit_tap(i)

    nc.finalize()
    return nc


def _get_nc():
    if "nc" not in _CACHE:
        _CACHE["nc"] = _build()
    return _CACHE["nc"]


def kernel_ext(inputs, trace=False):
    from concourse.bass_utils import run_bass_kernel_spmd

    nc = _get_nc()
    x = np.ascontiguousarray(inputs["x"], np.float32)
    common = {
        k: np.ascontiguousarray(inputs[k], np.float32)
        for k in ["Wq", "Wk", "Wv", "Wo", "bq", "bk", "bv", "bo", "gamma"]
    }
    in_maps = [{"xb": x[i], **common} for i in range(NCORES)]
    res = run_bass_kernel_spmd(nc, in_maps, list(range(NCORES)), trace=trace)
    out = np.stack([res.results[i]["out"] for i in range(NCORES)], axis=0)
    return out, res


def kernel(**inputs):
    out, _ = kernel_ext(inputs)
    return out



# revision 7
# speedup vs baseline: 1.2061x; 1.2061x over previous
"""Autoformer attention block kernel for 8 TRN2 NeuronCores.

Math reduction (validated vs reference to 2e-7):
 - output = x + AutoCorrelation(series_decomp(LN(x)))  (final decomp s2+t2 == x2)
 - mean over lags of the FFT cross-correlation == (sum_t Q)*(sum_t K)  (DC bin),
   so no FFT is needed: top-k stats come from column sums of `seasonal`.
 - column sums of seasonal need only the 48 boundary rows of LN(x) per batch
   (interior rows have zero net weight under I - movavg).
 - beta cancels exactly (band operator has row-sum 1); gamma folds into
   Wvo = diag(gamma) @ Wv @ Wo and the Wq/Wk row scaling.
 - delay aggregation = 64-tap circular FIR along time with data-dependent
   weights -> banded Toeplitz matmul on the TensorEngine.

Sharding: data-parallel over batch (B=8 -> 8 cores) with NO collective:
every core receives the 8*48 boundary rows of all batches (786KB) and
replicates the tiny top-40 selection locally, so cores run fully
independently (no rendezvous skew).
"""

import sys

if "/opt/trn_rl_repo" not in sys.path:
    sys.path.insert(0, "/opt/trn_rl_repo")

import numpy as np

L = 3072
D = 512
NT = L // 128  # 24 time tiles
H = 8
DK = 64
KTOP = 40
PAD = 12  # (25-1)//2
EPS = 1e-5
NCORES = 8
HL = float(H * L)

_CACHE = {}


def _np_consts():
    t = np.arange(L)
    lo = np.maximum(t - PAD, 0)
    hi = np.minimum(t + PAD + 1, L)
    inv = 1.0 / (hi - lo).astype(np.float64)

    # phi[s] = 1 - sum over t in the window around s of 1/win(t); nonzero only
    # in the first/last 24 positions.
    phi = np.ones(L, np.float64)
    for s in range(L):
        a = max(0, s - PAD)
        b = min(L, s + PAD + 1)
        phi[s] -= inv[a:b].sum()

    # band lhsT consts, all [128,128], K = a full z tile, zero-padded:
    # chunk X in {A: s = t0-128+j, B: s = t0+j, C: s = t0+128+j}:
    #   M[j, p] = delta(s, t0+p) - [|t0+p - s| <= PAD] / win(t0+p)
    def band(t0, soff):
        j = np.arange(128)[:, None]
        p = np.arange(128)[None, :]
        s = soff + j
        tp = t0 + p
        m = (np.abs(tp - s) <= PAD) & (s >= 0) & (s < L)
        M = -(m * inv[np.clip(tp, 0, L - 1)])
        M = M + (s == tp) * 1.0
        return np.ascontiguousarray(M, np.float32)

    t0m = 1280  # any interior tile
    b_A = band(t0m, t0m - 128)
    b_C = band(t0m, t0m + 128)
    b_Bf = band(0, 0)
    b_Bm = band(t0m, t0m)
    b_Bl = band(L - 128, L - 128)

    # PHI[row, b]: phi weight of boundary row `row` of xall48 toward batch b.
    # xall48 row layout: batch b occupies rows [48b, 48b+48): first 24 = x[b,:24],
    # last 24 = x[b, L-24:].
    PHI = np.zeros((384, 8), np.float32)
    for b in range(8):
        PHI[48 * b : 48 * b + 24, b] = phi[:24]
        PHI[48 * b + 24 : 48 * b + 48, b] = phi[-24:]

    ident = np.eye(128, dtype=np.float32)
    return b_A, b_C, b_Bf, b_Bm, b_Bl, PHI, ident


def _build():
    import concourse.bass as bass
    import concourse.tile as tile
    import concourse.mybir as mybir
    from concourse import bacc
    import bass_rust
    import ml_dtypes

    dt = mybir.dt
    f32 = dt.float32
    bf16 = dt.bfloat16
    AF = mybir.ActivationFunctionType
    ALU = mybir.AluOpType
    AX = mybir.AxisListType
    ts = bass.ts

    nc = bacc.Bacc(None, target_bir_lowering=False)

    xe = nc.dram_tensor("xb", [L, D], f32, kind="ExternalInput")
    x48e = nc.dram_tensor("xall48", [384, D], f32, kind="ExternalInput")
    bsele = nc.dram_tensor("bsel", [8, 1], f32, kind="ExternalInput")
    wqe = nc.dram_tensor("Wq", [D, D], f32, kind="ExternalInput")
    wke = nc.dram_tensor("Wk", [D, D], f32, kind="ExternalInput")
    wve = nc.dram_tensor("Wv", [D, D], f32, kind="ExternalInput")
    woe = nc.dram_tensor("Wo", [D, D], f32, kind="ExternalInput")
    bqe = nc.dram_tensor("bq", [D], f32, kind="ExternalInput")
    bke = nc.dram_tensor("bk", [D], f32, kind="ExternalInput")
    bve = nc.dram_tensor("bv", [D], f32, kind="ExternalInput")
    boe = nc.dram_tensor("bo", [D], f32, kind="ExternalInput")
    gme = nc.dram_tensor("gamma", [D], f32, kind="ExternalInput")
    oute = nc.dram_tensor("out", [L, D], f32, kind="ExternalOutput")

    bA, bC, bBf, bBm, bBl, PHI, ident = _np_consts()
    bf = ml_dtypes.bfloat16
    cbA = nc.inline_tensor(bA.astype(bf), "c_bA")
    cbC = nc.inline_tensor(bC.astype(bf), "c_bC")
    cbBf = nc.inline_tensor(bBf.astype(bf), "c_bBf")
    cbBm = nc.inline_tensor(bBm.astype(bf), "c_bBm")
    cbBl = nc.inline_tensor(bBl.astype(bf), "c_bBl")
    cPHI = nc.inline_tensor(PHI.astype(bf), "c_PHI")
    cid = nc.inline_tensor(ident.astype(bf), "c_id")
    cones1x64 = nc.inline_tensor(np.ones((1, 64), np.float32), "c_o64")
    cones1x128b = nc.inline_tensor(np.ones((1, 128), bf), "c_o128b")
    cones1x8 = nc.inline_tensor(np.ones((1, 8), bf), "c_o8")
    cones8x1 = nc.inline_tensor(np.ones((8, 1), np.float32), "c_o8x1")

    from contextlib import ExitStack

    with tile.TileContext(nc) as tc, ExitStack() as ctx:
        pc = ctx.enter_context(tc.tile_pool(name="consts", bufs=1))
        px = ctx.enter_context(tc.tile_pool(name="xarr", bufs=NT))
        pz = ctx.enter_context(tc.tile_pool(name="zroll", bufs=10))
        pvo = ctx.enter_context(tc.tile_pool(name="voarr", bufs=NT))
        pwvo = ctx.enter_context(tc.tile_pool(name="wvo", bufs=4))
        pwt = ctx.enter_context(tc.tile_pool(name="wtmp", bufs=4))
        pwork = ctx.enter_context(tc.tile_pool(name="work", bufs=3))
        psq = ctx.enter_context(tc.tile_pool(name="sqscr", bufs=2))
        pstt = ctx.enter_context(tc.tile_pool(name="stats", bufs=3))
        psm = ctx.enter_context(tc.tile_pool(name="smalls", bufs=2))
        pout = ctx.enter_context(tc.tile_pool(name="osb", bufs=3))
        pseasT = ctx.enter_context(tc.tile_pool(name="seasT", bufs=3))
        pdram = ctx.enter_context(tc.tile_pool(name="dram", bufs=1, space="DRAM"))
        qst = ctx.enter_context(tc.tile_pool(name="ps_st", bufs=1, space="PSUM"))
        qtp = ctx.enter_context(tc.tile_pool(name="ps_tp", bufs=1, space="PSUM"))
        qvo = ctx.enter_context(tc.tile_pool(name="ps_vo", bufs=2, space="PSUM"))
        qsm = ctx.enter_context(tc.tile_pool(name="ps_sm", bufs=2, space="PSUM"))
        qtap = ctx.enter_context(tc.tile_pool(name="ps_tap", bufs=2, space="PSUM"))

        # ---------------- constants to SBUF ----------------
        def cload(name, shape, src, dtype=f32):
            t = pc.tile(list(shape), dtype, tag=name)
            nc.sync.dma_start(t[:], src)
            return t

        idt = cload("idt", (128, 128), cid[:, :], bf16)
        gammaP = pc.tile([128, 4], f32, tag="gammaP")
        nc.sync.dma_start(gammaP[:], gme[:].rearrange("(a b) -> b a", b=128))
        bndA = cload("bndA", (128, 128), cbA[:, :], bf16)
        bndC = cload("bndC", (128, 128), cbC[:, :], bf16)
        bndBf = cload("bndBf", (128, 128), cbBf[:, :], bf16)
        bndBm = cload("bndBm", (128, 128), cbBm[:, :], bf16)
        bndBl = cload("bndBl", (128, 128), cbBl[:, :], bf16)
        o1x64 = cload("o1x64", (1, 64), cones1x64[:, :])
        o1x128b = cload("o1x128b", (1, 128), cones1x128b[:, :], bf16)
        o1x8 = cload("o1x8", (1, 8), cones1x8[:, :], bf16)
        o8x1f = cload("o8x1f", (8, 1), cones8x1[:, :])
        bvP = pc.tile([128, 4], f32, tag="bvP")
        nc.sync.dma_start(bvP[:], bve[:].rearrange("(a b) -> b a", b=128))
        bqv = pc.tile([1, 512], f32, tag="bqv")
        nc.sync.dma_start(bqv[:], bqe[:])
        bkv = pc.tile([1, 512], f32, tag="bkv")
        nc.sync.dma_start(bkv[:], bke[:])
        bov = pc.tile([1, 512], f32, tag="bov")
        nc.sync.dma_start(bov[:], boe[:])
        bq_sc = pc.tile([1, 512], bf16, tag="bq_sc")
        nc.scalar.mul(bq_sc[:], bqv[:], float(L))
        bk_sc = pc.tile([1, 512], bf16, tag="bk_sc")
        nc.scalar.mul(bk_sc[:], bkv[:], float(L))

        ones64 = nc.const_aps.tensor(1.0, (64, 1))

        # toeplitz scratch in DRAM ([128 x 192] p-major), zeroed early
        toep2d = pdram.tile([128, 192], bf16, tag="toep2d")
        zline = pc.tile([128, 192], bf16, tag="zline")
        nc.vector.memset(zline[:], 0.0)
        nc.sync.dma_start(toep2d[:], zline[:])
        wfd = pdram.tile([64], bf16, tag="wfd")

        # ---------------- x tiles + grouped LN stats ----------------
        xt = [None] * NT
        zt = [None] * NT

        def ln_tiles(tiles, xtiles, ztiles, st, zdst=None):
            """Row-wise LayerNorm: xtiles[i] -> ztiles[i] (bf16), stats in st."""
            n = len(tiles)
            for j, i in enumerate(tiles):
                nc.vector.tensor_reduce(
                    st[:, j : j + 1], xtiles[i][:], axis=AX.X, op=ALU.add
                )
                sq = psq.tile([128, 512], f32, tag="sq")
                nc.scalar.activation(
                    sq[:], xtiles[i][:], AF.Square, accum_out=st[:, 4 + j : 5 + j]
                )
            nc.vector.tensor_scalar(
                st[:, 8 : 8 + n], st[:, 0:n], 1.0 / D, None, op0=ALU.mult
            )
            nc.vector.tensor_tensor(
                st[:, 12 : 12 + n], st[:, 8 : 8 + n], st[:, 8 : 8 + n], op=ALU.mult
            )
            nc.vector.tensor_scalar(
                st[:, 16 : 16 + n], st[:, 4 : 4 + n], 1.0 / D, EPS,
                op0=ALU.mult, op1=ALU.add,
            )
            nc.vector.tensor_tensor(
                st[:, 20 : 20 + n], st[:, 16 : 16 + n], st[:, 12 : 12 + n],
                op=ALU.subtract,
            )
            nc.scalar.activation(st[:, 24 : 24 + n], st[:, 20 : 20 + n], AF.Sqrt)
            nc.vector.reciprocal(st[:, 28 : 28 + n], st[:, 24 : 24 + n])
            nc.vector.tensor_tensor(
                st[:, 32 : 32 + n], st[:, 8 : 8 + n], st[:, 28 : 28 + n],
                op=ALU.mult,
            )
            nc.vector.tensor_scalar(
                st[:, 32 : 32 + n], st[:, 32 : 32 + n], -1.0, None, op0=ALU.mult
            )
            for j, i in enumerate(tiles):
                z = (zdst or pz).tile([128, 512], bf16, tag="z")
                nc.scalar.activation(
                    z[:], xtiles[i][:], AF.Identity,
                    bias=st[:, 32 + j : 33 + j], scale=st[:, 28 + j : 29 + j],
                )
                ztiles[i] = z

        def emit_group(tiles):
            st = pstt.tile([128, 36], f32, tag="st")
            for i in tiles:
                x = px.tile([128, 512], f32, tag="x")
                nc.sync.dma_start(x[:], xe[ts(i, 128), :])
                xt[i] = x
            ln_tiles(tiles, xt, zt, st)

        # ---------------- seasonal (banded matmul) + vo ----------------
        vo = [None] * NT
        wvo = []  # filled by weight prep below
        toep_ref = {}

        def emit_seasonal(i):
            sps = qst.tile([128, 512], f32)
            if i == 0:
                chunks = [(bndBf[:], zt[0][:, :]), (bndC[:], zt[1][:, :])]
            elif i == NT - 1:
                chunks = [(bndA[:], zt[22][:, :]), (bndBl[:], zt[23][:, :])]
            else:
                chunks = [
                    (bndA[:], zt[i - 1][:, :]),
                    (bndBm[:], zt[i][:, :]),
                    (bndC[:], zt[i + 1][:, :]),
                ]
            nck = len(chunks)
            for k, (lt, rz) in enumerate(chunks):
                nc.tensor.matmul(
                    sps[:], lt, rz, start=(k == 0), stop=(k == nck - 1)
                )
            seas = pwork.tile([128, 512], bf16, tag="seas")
            nc.scalar.copy(seas[:], sps[:])
            tp = qtp.tile([128, 512], bf16, tag="tp")
            for c in range(4):
                nc.tensor.transpose(tp[:, ts(c, 128)], seas[:, ts(c, 128)], idt[:])
            sT = pseasT.tile([128, 512], bf16, tag="sT")
            nc.vector.tensor_copy(sT[:], tp[:])
            vps = qvo.tile([128, 512], f32)
            for c in range(4):
                nc.tensor.matmul(
                    vps[:], sT[:, ts(c, 128)], wvo[c][:],
                    start=(c == 0), stop=(c == 3),
                )
            v = pvo.tile([128, 512], bf16, tag="vo")
            nc.scalar.copy(v[:], vps[:])
            vo[i] = v

        # ---------------- tap + residual + output ----------------
        def emit_tap(i):
            toepA = toep_ref["A"]
            toepB = toep_ref["B"]
            cvb = toep_ref["cvb"]
            tps = qtap.tile([128, 512], f32)
            nc.tensor.matmul(tps[:], toepA[:], vo[i][:], start=True, stop=False)
            nc.tensor.matmul(
                tps[:], toepB[:], vo[(i + 1) % NT][0:63, :],
                start=False, stop=False,
            )
            nc.tensor.matmul(tps[:], o1x128b[:], cvb[:], start=False, stop=True)
            osb = pout.tile([128, 512], f32, tag="osb")
            nc.vector.tensor_tensor(osb[:], xt[i][:], tps[:], op=ALU.add)
            if i % 2 == 0:
                nc.scalar.dma_start(oute[ts(i, 128), :], osb[:])
            else:
                nc.sync.dma_start(oute[ts(i, 128), :], osb[:])

        # ---------------- mv: local top-k stats (no collective) ----------------
        # Every core computes mean_value for ALL 8 batches from the 48 boundary
        # rows of each batch (phi is nonzero only there), so the global top-40
        # selection needs no cross-core communication.
        with tc.high_priority():
            # Wq/Wk first (largest dependency of the toeplitz critical path),
            # scaled by gamma on cast (diag(gamma) @ Wq).
            wq_sb = []
            wk_sb = []
            for a in range(4):
                w = pwt.tile([128, 512], f32, tag="wq")
                nc.sync.dma_start(w[:], wqe[ts(a, 128), :])
                wb = pwt.tile([128, 512], bf16, tag="wqb")
                nc.scalar.activation(wb[:], w[:], AF.Identity, scale=gammaP[:, a : a + 1])
                wq_sb.append(wb)
                w = pwt.tile([128, 512], f32, tag="wk")
                nc.sync.dma_start(w[:], wke[ts(a, 128), :])
                wb = pwt.tile([128, 512], bf16, tag="wkb")
                nc.scalar.activation(wb[:], w[:], AF.Identity, scale=gammaP[:, a : a + 1])
                wk_sb.append(wb)

            phiT = []
            x48t = [None] * 3
            z48t = [None] * 3
            for k in range(3):
                x = pwork.tile([128, 512], f32, tag="x48")
                nc.sync.dma_start(x[:], x48e[ts(k, 128), :])
                x48t[k] = x
                p = pc.tile([128, 8], bf16, tag=f"phiT{k}")
                nc.sync.dma_start(p[:], cPHI[ts(k, 128), :])
                phiT.append(p)
            st48 = pstt.tile([128, 36], f32, tag="st48")
            ln_tiles([0, 1, 2], x48t, z48t, st48, zdst=pwork)

            # ssT[d, b] = sum_row z48[row, d] * PHI[row, b]   (4 d-chunks)
            ssb = psm.tile([128, 32], bf16, tag="ssb")
            for c in range(4):
                ssps = qsm.tile([128, 8], f32, tag="sm")
                for k in range(3):
                    nc.tensor.matmul(
                        ssps[:], z48t[k][:, ts(c, 128)], phiT[k][:],
                        start=(k == 0), stop=(k == 2),
                    )
                nc.vector.tensor_copy(ssb[:, c * 8 : c * 8 + 8], ssps[:])

            # Qs[b, :] = ss[b, :] @ (diag(gamma) Wq) + L*bq ; same for Ks
            qs_ps = qsm.tile([8, 512], f32, tag="sm")
            for c in range(4):
                nc.tensor.matmul(
                    qs_ps[:], ssb[:, c * 8 : c * 8 + 8], wq_sb[c][:],
                    start=(c == 0), stop=False,
                )
            nc.tensor.matmul(qs_ps[:], o1x8[:], bq_sc[:], start=False, stop=True)
            ks_ps = qsm.tile([8, 512], f32, tag="sm")
            for c in range(4):
                nc.tensor.matmul(
                    ks_ps[:], ssb[:, c * 8 : c * 8 + 8], wk_sb[c][:],
                    start=(c == 0), stop=False,
                )
            nc.tensor.matmul(ks_ps[:], o1x8[:], bk_sc[:], start=False, stop=True)

            qsv = psm.tile([8, 512], f32, tag="qsv")
            nc.scalar.copy(qsv[:], qs_ps[:])
            pr = psm.tile([8, 512], f32, tag="pr")
            nc.vector.tensor_tensor(pr[:], qsv[:], ks_ps[:], op=ALU.mult)
            mvr = psm.tile([8, 64], f32, tag="mvr")
            nc.vector.tensor_reduce(
                mvr[:], pr[:].rearrange("p (h c) -> p c h", h=H),
                axis=AX.X, op=ALU.add,
            )
            mv_all = psm.tile([8, 64], f32, tag="mv")
            nc.scalar.mul(mv_all[:], mvr[:], 1.0 / HL)

            # global ranking row (sum over batches) + own-batch row
            bsel_sb = psm.tile([8, 1], f32, tag="bsel")
            nc.sync.dma_start(bsel_sb[:], bsele[:, :])
            # g_row[0, d] = sum_b mv[b, d]  (free-major)
            grow_ps = qsm.tile([1, 64], f32, tag="sm")
            nc.tensor.matmul(grow_ps[:], o8x1f[:], mv_all[:], start=True, stop=True)
            g_row = psm.tile([1, 64], f32, tag="grow")
            nc.vector.tensor_copy(g_row[:], grow_ps[:])
            # mvP[d, 0] = mv[own_batch, d]  (partition-major)
            mvP_ps = qsm.tile([64, 1], f32, tag="sm")
            nc.tensor.matmul(mvP_ps[:], mv_all[:], bsel_sb[:], start=True, stop=True)
            mvP = psm.tile([64, 1], f32, tag="mvP")
            nc.vector.tensor_copy(mvP[:], mvP_ps[:])
            # gP[d, 0] = g_row[0, d]  (partition-major)
            gP_ps = qsm.tile([64, 1], f32, tag="sm")
            nc.tensor.matmul(
                gP_ps[:], g_row[:], nc.const_aps.tensor(1.0, (1, 1)),
                start=True, stop=True,
            )
            gP = psm.tile([64, 1], f32, tag="gP")
            nc.vector.tensor_copy(gP[:], gP_ps[:])

            # gf[j, d] = g[d] for all j: outer(ones64, g_row)
            gf_ps = qsm.tile([64, 64], f32, tag="sm")
            nc.tensor.matmul(gf_ps[:], o1x64[:], g_row[:], start=True, stop=True)

            sc = psm.tile([64, 8], f32, tag="scm")
            cmp = psm.tile([64, 64], f32, tag="cmp")
            # cmp[j, d] = (g[d] > g[j]); rank[j] = row-sum; keep rank < 39.5
            nc.vector.tensor_tensor(
                cmp[:], gf_ps[:], gP[:].to_broadcast((64, 64)), op=ALU.is_gt
            )
            nc.vector.tensor_reduce(sc[:, 0:1], cmp[:], axis=AX.X, op=ALU.add)
            nc.vector.tensor_scalar(
                sc[:, 1:2], sc[:, 0:1], KTOP - 0.5, None, op0=ALU.is_lt
            )
            nc.scalar.activation(sc[:, 2:3], mvP[:], AF.Exp)
            nc.vector.tensor_tensor(sc[:, 3:4], sc[:, 2:3], sc[:, 1:2], op=ALU.mult)
            s_ps = qsm.tile([1, 1], f32, tag="sm")
            nc.tensor.matmul(s_ps[:], sc[:, 3:4], ones64, start=True, stop=True)
            rs = psm.tile([1, 1], f32, tag="rs")
            nc.vector.reciprocal(rs[:], s_ps[:])
            rsf_ps = qsm.tile([64, 1], f32, tag="sm")
            nc.tensor.matmul(rsf_ps[:], o1x64[:], rs[:], start=True, stop=True)
            wf = psm.tile([64, 1], f32, tag="wf")
            nc.vector.tensor_tensor(wf[:], sc[:, 3:4], rsf_ps[:], op=ALU.mult)
            wfb = psm.tile([64, 1], bf16, tag="wfb")
            nc.vector.tensor_copy(wfb[:], wf[:])

            # toeplitz build: wf -> DRAM -> burst scatter (64-elem runs) ->
            # [p, j] tile -> PE transpose -> lhsT tiles toepA/toepB
            nc.sync.dma_start(wfd[:], wfb[:])
            dst = toep2d[:].flatten()
            dst.ap = bass_rust.VecI64Pair([[193, 128], [1, 64]])
            src = wfd[:].flatten()
            src.ap = bass_rust.VecI64Pair([[0, 128], [1, 64]])
            nc.sync.dma_start(dst, src)
            t2sb = pc.tile([128, 192], bf16, tag="t2sb")
            nc.sync.dma_start(t2sb[:], toep2d[:])
            tpa = qtp.tile([128, 128], bf16, tag="tp")
            nc.tensor.transpose(tpa[:], t2sb[:, 0:128], idt[:])
            toepA = pc.tile([128, 128], bf16, tag="toepA")
            nc.vector.tensor_copy(toepA[:], tpa[:])
            tpb = qtp.tile([128, 128], bf16, tag="tp")
            nc.tensor.transpose(tpb[0:63, :], t2sb[:, 128:191], idt[:])
            toepB = pc.tile([63, 128], bf16, tag="toepB")
            nc.vector.tensor_copy(toepB[:], tpb[0:63, :])
            toep_ref["A"] = toepA
            toep_ref["B"] = toepB

        # ---------------- weight prep: Wvo = diag(gamma) Wv Wo, cvec ----------------
        wo_sb = []
        for a in range(4):
            w = pwt.tile([128, 512], f32, tag="wo")
            nc.sync.dma_start(w[:], woe[ts(a, 128), :])
            wo_sb.append(w)
        wob = []
        for a in range(4):
            w = pwt.tile([128, 512], bf16, tag="wob")
            nc.vector.tensor_copy(w[:], wo_sb[a][:])
            wob.append(w)
        wv_sc = []
        for a in range(4):
            w = pwt.tile([128, 512], f32, tag="wv")
            nc.sync.dma_start(w[:], wve[ts(a, 128), :])
            ws = pwt.tile([128, 512], bf16, tag="wvs")
            nc.scalar.activation(ws[:], w[:], AF.Identity, scale=gammaP[:, a : a + 1])
            wv_sc.append(ws)
        wvT = []
        for c in range(4):
            w = pwt.tile([128, 512], bf16, tag="wvT")
            wvT.append(w)
        for a in range(4):
            for c in range(4):
                tp = qtp.tile([128, 128], bf16, tag="tp")
                nc.tensor.transpose(tp[:], wv_sc[a][:, ts(c, 128)], idt[:])
                nc.vector.tensor_copy(wvT[c][:, ts(a, 128)], tp[:])
        for a in range(4):
            vps = qvo.tile([128, 512], f32)
            for c in range(4):
                nc.tensor.matmul(
                    vps[:], wvT[c][:, ts(a, 128)], wob[c][:],
                    start=(c == 0), stop=(c == 3),
                )
            w = pwvo.tile([128, 512], bf16, tag="wvo")
            nc.scalar.copy(w[:], vps[:])
            wvo.append(w)

        # cvec = bv @ Wo + bo (bf16 row for the tap matmul)
        cps = qsm.tile([1, 512], f32, tag="sm")
        for c in range(4):
            nc.tensor.matmul(
                cps[:], bvP[:, c : c + 1], wo_sb[c][:],
                start=(c == 0), stop=(c == 3),
            )
        cv_sb = psm.tile([1, 512], f32, tag="cv")
        nc.vector.tensor_tensor(cv_sb[:], cps[:], bov[:], op=ALU.add)
        cvb = psm.tile([1, 512], bf16, tag="cvb")
        nc.vector.tensor_copy(cvb[:], cv_sb[:])
        toep_ref["cvb"] = cvb

        # ---------------- main pipeline ----------------
        groups = [[0, 1, 2, 3], [4, 5, 6, 7], [8, 9, 10, 11],
                  [12, 13, 14, 15], [16, 17, 18, 19], [20, 21, 22, 23]]
        state = {"sea": 0, "tap": 0}

        def advance():
            while state["sea"] < NT:
                i = state["sea"]
                need = [n for n in (i - 1, i, i + 1) if 0 <= n < NT]
                if not all(zt[n] is not None for n in need):
                    break
                emit_seasonal(i)
                state["sea"] += 1
            # taps: tap i needs vo[i] and vo[(i+1)%NT]; tap NT-1 needs vo[0]
            while state["tap"] < NT - 1:
                i = state["tap"]
                if vo[i] is None or vo[i + 1] is None:
                    break
                emit_tap(i)
                state["tap"] += 1
            if state["sea"] == NT and state["tap"] == NT - 1:
                emit_tap(NT - 1)
                state["tap"] = NT

        for g in groups:
            emit_group(g)
            advance()

    nc.finalize()
    return nc


def _get_nc():
    if "nc" not in _CACHE:
        _CACHE["nc"] = _build()
    return _CACHE["nc"]


def kernel_ext(inputs, trace=False):
    from concourse.bass_utils import run_bass_kernel_spmd

    nc = _get_nc()
    x = np.ascontiguousarray(inputs["x"], np.float32)
    xall48 = np.ascontiguousarray(
        np.concatenate(
            [np.concatenate([x[b, :24], x[b, L - 24 :]], axis=0) for b in range(NCORES)],
            axis=0,
        ),
        np.float32,
    )
    common = {
        k: np.ascontiguousarray(inputs[k], np.float32)
        for k in ["Wq", "Wk", "Wv", "Wo", "bq", "bk", "bv", "bo", "gamma"]
    }
    common["xall48"] = xall48
    in_maps = []
    for i in range(NCORES):
        bsel = np.zeros((8, 1), np.float32)
        bsel[i, 0] = 1.0
        in_maps.append({"xb": x[i], "bsel": bsel, **common})
    res = run_bass_kernel_spmd(nc, in_maps, list(range(NCORES)), trace=trace)
    out = np.stack([res.results[i]["out"] for i in range(NCORES)], axis=0)
    return out, res


def kernel(**inputs):
    out, _ = kernel_ext(inputs)
    return out
